# revision 24
# baseline (speedup 1.0000x reference)
"""Multi-head attention (B=512,S=64,D=1024,H=16) on 8 trn2 NeuronCores.

Strategy: pure data-parallel over the batch dim — each core gets 64 batches
(4096 tokens) and runs the full fused MHA layer locally; no collectives.

Per-core dataflow (token chunks of 512 = 8 batches):
  x [tok,1024] --PE transpose--> xT [1024,tok] (feature-major, bf16)
               --ACT scale-cast--> xT8 fp8 pair tiles [128,2,512] (x*16)
  qT/kT = fp8 DoubleRow GEMM (W*16 fp8 pair tiles), DVE evac scale 1/256
          -> bf16 feature-major m-tiles [128,512]
  v  = x @ Wv bf16 (token-major, interleaved with ones col in vaug)
  scoresT[k,q]: per (batch-pair u, t-pair) two psum banks, one per head
          parity (row-strip), each holding two heads' scores -> ONE exp
          ACT [128,128] per bank (exp/copy/identity share an ACT table;
          gelu does not, so gelus are clustered at chunk end)
  ctx[q,:]|sumexp[q] = expS.T @ [v|1] -> merged recip + one broadcast-mult TT
  ctxT via PE transpose; out = gelu(ctx @ Wo) accumulated token-major -> DRAM

fp8 notes: e4m3 needs pre-scaling (x16) to stay out of the subnormal range;
Q/K tolerate plain fp8 (softmax normalizes the small logit error), V/O do
not (W-quantization error hits the output directly), so V/O stay bf16.
DoubleRow sustains ~1.9x bf16 row throughput on HW. fp8 casts run on ACT
(Copy w/ scale); DVE/GPSIMD do fp8 stores 5-50x slower.

PSUM rules (hardware, probe-verified): concurrent matmuls may share a bank
only on the same row-strip or a strict diagonal; start=True zeroes the full
bank for the partitions it writes, so a second quadrant write on the same
partitions uses start=False, and sibling accumulation groups on the same
partitions must be strictly sequential (h-outer ordering in the DR GEMMs).
"""

import sys

sys.path.insert(0, "/opt/trn_rl_repo")

import numpy as np

import concourse.bass as bass
import concourse.tile as tile
from concourse import mybir
from concourse.bass_utils import run_bass_kernel_spmd
from concourse.masks import make_identity

F32 = mybir.dt.float32
BF = mybir.dt.bfloat16
F8 = mybir.dt.float8e4

B, S, D, H = 512, 64, 1024, 16
DH = D // H  # 64
NCORES = 8
BL = B // NCORES  # 64 batches per core
NTOK = BL * S  # 4096 tokens per core
CHUNK = 512  # tokens per pipeline chunk (8 batches)
NCH = NTOK // CHUNK  # 8
TT = CHUNK // 128  # 4 token-tiles per chunk
KT = D // 128  # 8 d-tiles
NP = KT // 2  # 4 fp8 k-pair tiles
SCALE = 1.0 / np.sqrt(np.float32(D))  # 1/32
XS = 16.0  # fp8 pre-scale for x
WS = 16.0  # fp8 pre-scale for wq/wk


def _split_multiwait(nc, limit=1):
    """walrus can emit at most one sync-wait per instruction; TileContext's
    tail drain carries one wait per touched processor. Hoist extras onto
    chained NOPs."""
    f = nc.m.functions[0]
    for blk in f.blocks:
        new_insts = []
        for inst in blk.instructions:
            si = inst.sync_info
            if si is not None and len(si.on_wait) > limit:
                extra = si.on_wait[:-limit]
                keep = si.on_wait[-limit:]
                for i, w in enumerate(extra):
                    nop = mybir.InstNoOp(
                        name=f"{inst.name}-waitsplit{i}",
                        sync_info=mybir.SyncInfo(on_wait=[w], on_update=[]),
                        bass_nofuse=True,
                        ins=[],
                        outs=[],
                    )
                    nop.engine = inst.engine
                    new_insts.append(nop)
                si.on_wait[:] = keep
            new_insts.append(inst)
        blk.instructions[:] = new_insts


def _interleave(a, b):
    """Merge two unit lists round-robin, proportionally to their lengths."""
    out = []
    ia = ib = 0
    la, lb = len(a), len(b)
    while ia < la or ib < lb:
        if ib >= lb or (ia < la and ia * lb <= ib * la):
            out.append(a[ia])
            ia += 1
        else:
            out.append(b[ib])
            ib += 1
    return out


def build(with_bias=False, split_waits=True):
    nc = bass.Bass("TRN2", debug=False, num_devices=NCORES)

    x_d = nc.declare_dram_parameter("x", [NTOK, D], F32, isOutput=False)
    w_d = {}
    b_d = {}
    for nm in ("wq", "wk", "wv", "wo"):
        w_d[nm] = nc.declare_dram_parameter(f"{nm}_w", [D, D], F32, isOutput=False)
        b_d[nm] = nc.declare_dram_parameter(f"{nm}_b", [D], F32, isOutput=False)
    out_d = nc.declare_dram_parameter("out", [NTOK, D], F32, isOutput=True)

    with tile.TileContext(nc) as tc:
        with (
            tc.tile_pool(name="weights", bufs=1) as wpool,
            tc.tile_pool(name="consts", bufs=1) as cpool,
            tc.tile_pool(name="wload", bufs=2) as ldpool,
            tc.tile_pool(name="xin", bufs=2) as xpool,
            tc.tile_pool(name="feat", bufs=2) as fpool,
            tc.tile_pool(name="attn", bufs=4) as apool,
            tc.tile_pool(name="outb", bufs=2) as opool,
            tc.tile_pool(name="psum", bufs=2, space="PSUM") as ppool,
        ):
            # fp8 pair tiles for Q/K: w8[which][p] = [128, 2, 8m, 128]
            w8 = {nm: [None] * NP for nm in ("wq", "wk")}
            # bf16 weights for V/O: wt[nm][k] = [128, D]
            wt = {nm: [None] * KT for nm in ("wv", "wo")}
            biases = {}
            consts = {}

            def pe_transpose(src, dst, evac=None):
                ps = ppool.tile([128, 128], BF, tag="tp", bufs=1, name="ps_tp")
                nc.tensor.transpose(ps, src, consts["identity"])
                (evac or nc.vector).tensor_copy(out=dst, in_=ps)

            def unit_load_w8(nm, k):
                """Load f32 ktile of wq/wk, ACT scale-cast into fp8 pair tiles."""
                def f():
                    wf = ldpool.tile([128, D], F32, tag="wload", name="wf")
                    nc.sync.dma_start(
                        out=wf[:], in_=w_d[nm][k * 128 : (k + 1) * 128, :]
                    )
                    if w8[nm][k // 2] is None:
                        w8[nm][k // 2] = wpool.tile(
                            [128, 2, KT, 128], F8, tag=f"w8_{nm}_{k // 2}",
                            name=f"w8{nm}{k // 2}",
                        )
                    nc.scalar.activation(
                        out=w8[nm][k // 2][:, k % 2, :, :],
                        in_=wf[:],
                        func=mybir.ActivationFunctionType.Copy,
                        scale=float(WS),
                    )

                return f

            def unit_load_w16(nm, k):
                def f():
                    wf = ldpool.tile([128, D], F32, tag="wload", name="wf")
                    nc.sync.dma_start(
                        out=wf[:], in_=w_d[nm][k * 128 : (k + 1) * 128, :]
                    )
                    wb = wpool.tile([128, D], BF, tag=f"w_{nm}_{k}", name=f"w{nm}{k}")
                    nc.vector.tensor_copy(out=wb[:], in_=wf[:])
                    wt[nm][k] = wb

                return f

            def unit_qk_biases():
                def f():
                    for nm in ("wq", "wk"):
                        bt = cpool.tile([128, KT], F32, tag=f"{nm}_pb", name=f"{nm}_pb")
                        nc.sync.dma_start(
                            out=bt[:], in_=b_d[nm][:].rearrange("(m p) -> p m", p=128)
                        )
                        biases[nm] = bt

                return f

            def unit_biases():
                """Only used in the with_bias build: v/o biases folded into a
                rank-1 bias_row = wv_b @ Wo + wo_b added inside the O GEMM."""
                def f():
                    vb = cpool.tile([128, KT], F32, tag="vb_pb", name="vb_pb")
                    nc.sync.dma_start(
                        out=vb[:], in_=b_d["wv"][:].rearrange("(m p) -> p m", p=128)
                    )
                    vbb = cpool.tile([128, KT], BF, tag="vb_pbb", name="vb_pbb")
                    nc.vector.tensor_copy(out=vbb[:], in_=vb[:])
                    ob = ldpool.tile([1, D], F32, tag="wload", name="ob")
                    nc.sync.dma_start(out=ob[:], in_=b_d["wo"][:].unsqueeze(0))
                    brow = cpool.tile([1, D], BF, tag="brow", name="brow")
                    for n in range(2):
                        psb = ppool.tile([1, 512], F32, tag="cx", bufs=2, name="psb")
                        for k in range(KT):
                            nc.tensor.matmul(
                                psb[:],
                                lhsT=vbb[:, k : k + 1],
                                rhs=wt["wo"][k][:, n * 512 : (n + 1) * 512],
                                start=(k == 0),
                                stop=(k == KT - 1),
                            )
                        nc.vector.tensor_tensor(
                            out=brow[:, n * 512 : (n + 1) * 512],
                            in0=psb[:],
                            in1=ob[:, n * 512 : (n + 1) * 512],
                            op=mybir.AluOpType.add,
                        )
                    biases["brow"] = brow
                    ones_col = cpool.tile([1, 128], BF, tag="ones_col", name="ones_col")
                    nc.gpsimd.memset(ones_col[:], 1.0)
                    biases["ones_col"] = ones_col

                return f

            live = {}  # per-chunk tiles handed from stage A to stage B

            def stage_a_units(ch):
                """X load + transpose + fp8 prep, then QKV projections."""
                tok0 = ch * CHUNK
                st = live.setdefault(ch, {})

                def u_x(t):
                    def f():
                        if "xT" not in st:
                            st["xT"] = fpool.tile(
                                [128, KT, CHUNK], BF, tag="xT", name="xT"
                            )
                        xf = xpool.tile([128, D], F32, tag="xf32", name="xf")
                        nc.sync.dma_start(
                            out=xf[:], in_=x_d[tok0 + t * 128 : tok0 + (t + 1) * 128, :]
                        )
                        xb = xpool.tile([128, D], BF, tag="xbf", name="xb")
                        nc.gpsimd.tensor_copy(out=xb[:], in_=xf[:])
                        for k in range(KT):
                            pe_transpose(
                                xb[:, k * 128 : (k + 1) * 128],
                                st["xT"][:, k, t * 128 : (t + 1) * 128],
                            )

                    return f

                def u_x8(p):
                    def f():
                        if "xT8" not in st:
                            st["xT8"] = [
                                fpool.tile(
                                    [128, 2, CHUNK], F8, tag=f"xT8{i}", name=f"xT8{i}"
                                )
                                for i in range(NP)
                            ]
                        for i in range(2):
                            nc.scalar.activation(
                                out=st["xT8"][p][:, i, :],
                                in_=st["xT"][:, 2 * p + i, :],
                                func=mybir.ActivationFunctionType.Copy,
                                scale=float(XS),
                            )

                    return f

                def u_qk(which, m):
                    def f():
                        key = "qT" if which == "wq" else "kT"
                        if key not in st:
                            st[key] = [
                                fpool.tile([128, CHUNK], BF, tag=f"{key}{i}",
                                           name=f"{key}{i}")
                                for i in range(KT)
                            ]
                        ps = ppool.tile([128, CHUNK], F32, tag="proj", bufs=3, name="ps_qk")
                        for h in range(2):
                            for p in range(NP):
                                nc.tensor.matmul(
                                    ps[:, h * 256 : (h + 1) * 256],
                                    lhsT=w8[which][p][:, :, m, :],
                                    rhs=st["xT8"][p][:, :, h * 256 : (h + 1) * 256],
                                    start=(p == 0),
                                    stop=(p == NP - 1),
                                    perf_mode=mybir.MatmulPerfMode.DoubleRow,
                                )
                        if with_bias:
                            nc.scalar.activation(
                                out=st[key][m][:],
                                in_=ps[:],
                                func=mybir.ActivationFunctionType.Identity,
                                scale=float(1.0 / (XS * WS)),
                                bias=biases[which][:, m : m + 1],
                            )
                        else:
                            nc.scalar.activation(
                                out=st[key][m][:],
                                in_=ps[:],
                                func=mybir.ActivationFunctionType.Copy,
                                scale=float(1.0 / (XS * WS)),
                            )

                    return f

                def u_v(t, n):
                    def f():
                        if "vaug" not in st:
                            st["vaug"] = [
                                apool.tile(
                                    [128, H * (DH + 1)], BF,
                                    tag=f"vaug{i}", name=f"vaug{i}", bufs=2,
                                )
                                for i in range(TT)
                            ]
                            for i in range(TT):
                                nc.gpsimd.memset(
                                    st["vaug"][i][:]
                                    .rearrange("p (h c) -> p h c", c=DH + 1)[:, :, DH : DH + 1],
                                    1.0,
                                )
                        ps = ppool.tile([128, CHUNK], F32, tag="proj", bufs=3, name="ps_v")
                        for k in range(KT):
                            nc.tensor.matmul(
                                ps[:],
                                lhsT=st["xT"][:, k, t * 128 : (t + 1) * 128],
                                rhs=wt["wv"][k][:, n * 512 : (n + 1) * 512],
                                start=(k == 0),
                                stop=(k == KT - 1),
                            )
                        nc.vector.tensor_copy(
                            out=st["vaug"][t][:]
                            .rearrange("p (h c) -> p h c", c=DH + 1)[:, n * 8 : (n + 1) * 8, 0:DH],
                            in_=ps[:].rearrange("p (j c) -> p j c", c=DH),
                        )

                    return f

                proj = []
                for m in range(KT):
                    proj.append(u_qk("wq", m))
                    proj.append(u_qk("wk", m))
                for t in range(TT):
                    for n in range(2):
                        proj.append(u_v(t, n))
                return {
                    "x": [u_x(t) for t in range(TT)],
                    "x8": [u_x8(p) for p in range(NP)],
                    "q": [u_qk("wq", m) for m in range(KT)],
                    "k": [u_qk("wk", m) for m in range(KT)],
                    "v": [u_v(t, n) for t in range(TT) for n in range(2)],
                    "proj": proj,
                }

            def attn_units(ch):
                """Attention for chunk ch. Scores for a (u, t-pair) fill two
                psum banks (one per head parity = row strip), two heads per
                bank, one exp each. Gelu/out units are clustered at the end
                of the chunk to avoid ACT table thrash."""
                st = live[ch]
                es_tiles = {}

                def u_scores(u, tq):
                    def f():
                        qT, kT = st["qT"], st["kT"]
                        sc = {}
                        for hh in (0, 1):
                            sc[hh] = ppool.tile(
                                [128, 256], F32, tag=f"sc{hh}",
                                bufs=1, name=f"ps_sc{hh}",
                            )
                        for hh in (0, 1):
                            hsl = slice(hh * 64, hh * 64 + 64)
                            for dt in range(4):
                                t = 4 * tq + dt
                                for bpar in (0, 1):
                                    toksl = slice(
                                        u * 128 + bpar * 64, u * 128 + bpar * 64 + 64
                                    )
                                    nc.tensor.matmul(
                                        sc[hh][
                                            bpar * 64 : bpar * 64 + 64,
                                            dt * 64 : dt * 64 + 64,
                                        ],
                                        lhsT=kT[t][hsl, toksl],
                                        rhs=qT[t][hsl, toksl],
                                        start=(dt == 0),
                                        stop=(dt == 3),
                                        skip_group_check=True,
                                    )
                        for hh in (0, 1):
                            es = apool.tile([128, 256], BF, tag="expS", name="es", bufs=8)
                            es_tiles[(u, tq, hh)] = es
                            nc.scalar.activation(
                                out=es[:],
                                in_=sc[hh][:],
                                func=mybir.ActivationFunctionType.Exp,
                                scale=float(SCALE),
                            )

                    return f

                def u_ctx(u, t):
                    def f():
                        if "ctx" not in st:
                            st["ctx"] = [
                                apool.tile([128, D], BF, tag=f"ctx{i}", name=f"ctx{i}", bufs=2)
                                for i in range(TT)
                            ]
                        vaug, ctx = st["vaug"], st["ctx"]
                        tq, dt = t // 4, t % 4
                        ps_c = ppool.tile([128, 130], F32, tag="cx", bufs=2, name="ps_c")
                        for bpar in (0, 1):
                            bsl = slice(bpar * 64, bpar * 64 + 64)
                            for hh in (0, 1):
                                h = 2 * t + hh
                                es = es_tiles[(u, tq, hh)]
                                nc.tensor.matmul(
                                    ps_c[bsl, hh * 65 : hh * 65 + 65],
                                    lhsT=es[bsl, dt * 64 : dt * 64 + 64],
                                    rhs=vaug[u][bsl, h * 65 : (h + 1) * 65],
                                    start=True,
                                    stop=True,
                                )
                        if dt == 3:
                            for hh in (0, 1):
                                es_tiles.pop((u, tq, hh))
                        rc2 = apool.tile([128, 2], F32, tag="recip", name="rc2")
                        psv = ps_c[:].rearrange("p (h c) -> p h c", c=65)
                        nc.vector.reciprocal(rc2[:], psv[:, :, DH])
                        nc.vector.tensor_tensor(
                            out=ctx[u][:, 2 * t * DH : (2 * t + 2) * DH].rearrange(
                                "p (h c) -> p h c", c=DH
                            ),
                            in0=psv[:, :, 0:DH],
                            in1=rc2[:, :, None].to_broadcast([128, 2, DH]),
                            op=mybir.AluOpType.mult,
                        )

                    return f

                def u_ctxT(u):
                    def f():
                        if "cT" not in st:
                            st["cT"] = fpool.tile(
                                [128, KT, CHUNK], BF, tag="cT", name="cT"
                            )
                        for k in range(KT):
                            pe_transpose(
                                st["ctx"][u][:, k * 128 : (k + 1) * 128],
                                st["cT"][:, k, u * 128 : (u + 1) * 128],
                            )

                    return f

                def u_out(t, n):
                    tok0 = ch * CHUNK

                    def f():
                        ps = ppool.tile([128, CHUNK], F32, tag="proj", bufs=3, name="ps_o")
                        for k in range(KT):
                            nc.tensor.matmul(
                                ps[:],
                                lhsT=st["cT"][:, k, t * 128 : (t + 1) * 128],
                                rhs=wt["wo"][k][:, n * 512 : (n + 1) * 512],
                                start=(k == 0),
                                stop=(k == KT - 1) and not with_bias,
                            )
                        if with_bias:
                            nc.tensor.matmul(
                                ps[:],
                                lhsT=biases["ones_col"][:],
                                rhs=biases["brow"][:, n * 512 : (n + 1) * 512],
                                start=False,
                                stop=True,
                            )
                        og = opool.tile([128, 512], F32, tag="ogelu", name="og", bufs=4)
                        nc.scalar.activation(
                            out=og[:], in_=ps[:], func=mybir.ActivationFunctionType.Gelu
                        )
                        nc.sync.dma_start(
                            out=out_d[
                                tok0 + t * 128 : tok0 + (t + 1) * 128,
                                n * 512 : (n + 1) * 512,
                            ],
                            in_=og[:],
                        )

                    return f

                units = []
                for u in range(TT):
                    units.append(u_scores(u, 0))
                    if u > 0:
                        units.append(u_ctxT(u - 1))
                    units.append(u_ctx(u, 0))
                    units.append(u_ctx(u, 1))
                    units.append(u_scores(u, 1))
                    for t in range(2, KT):
                        units.append(u_ctx(u, t))
                units.append(u_ctxT(TT - 1))
                # gelu cluster: all out-projections of the chunk back to back
                for u in range(TT):
                    units.append(u_out(u, 0))
                    units.append(u_out(u, 1))
                return units

            # ---- emission ----
            identity = cpool.tile([128, 128], BF, tag="ident", name="identity")
            make_identity(nc, identity[:])
            consts["identity"] = identity
            stages = [stage_a_units(ch) for ch in range(NCH)]
            # prologue: x(0), wq8 loads, x8(0), then chunk-0 Q interleaved with
            # wk8 loads and x(1); K with wv loads; V with wo loads.
            for t in range(TT):
                stages[0]["x"][t]()
            for p in range(NP):
                stages[0]["x8"][p]()
            if with_bias:
                unit_qk_biases()()
            for k in range(KT):
                unit_load_w8("wq", k)()
            for u in _interleave(
                _interleave(stages[0]["q"], stages[1]["x"][:2]),
                [unit_load_w8("wk", k) for k in range(KT)],
            ):
                u()
            for u in _interleave(
                _interleave(stages[0]["k"], stages[1]["x"][2:]),
                [unit_load_w16("wv", k) for k in range(KT)],
            ):
                u()
            for u in _interleave(
                stages[0]["v"],
                [unit_load_w16("wo", k) for k in range(KT)],
            ):
                u()
            if with_bias:
                unit_biases()()
            # steady state: block ch emits x8(ch) + proj(ch) + x(ch+1) + attn(ch-1)
            for ch in range(1, NCH):
                dense = stages[ch]["x8"] + stages[ch]["proj"]
                if ch + 1 < NCH:
                    dense = _interleave(dense, stages[ch + 1]["x"])
                for u in _interleave(dense, attn_units(ch - 1)):
                    u()
                live.pop(ch - 1)
            for u in attn_units(NCH - 1):
                u()
            live.pop(NCH - 1)

    if split_waits:
        _split_multiwait(nc)
    return nc


_NC = {}


def _get_nc(with_bias):
    if with_bias not in _NC:
        _NC[with_bias] = build(with_bias=with_bias)
    return _NC[with_bias]


def _make_in_maps(inputs):
    x = np.ascontiguousarray(np.asarray(inputs["x"], dtype=np.float32))
    full = {
        nm: np.ascontiguousarray(np.asarray(inputs[nm], dtype=np.float32))
        for nm in ("wq_w", "wq_b", "wk_w", "wk_b", "wv_w", "wv_b", "wo_w", "wo_b")
    }
    in_maps = []
    for c in range(NCORES):
        m = {"x": np.ascontiguousarray(x[c * BL : (c + 1) * BL].reshape(NTOK, D))}
        m.update(full)
        in_maps.append(m)
    return in_maps


def _with_bias(inputs):
    return any(
        np.abs(np.asarray(inputs[nm])).max() > 0
        for nm in ("wq_b", "wk_b", "wv_b", "wo_b")
    )


def kernel(**inputs):
    nc = _get_nc(_with_bias(inputs))
    res = run_bass_kernel_spmd(
        nc, _make_in_maps(inputs), core_ids=list(range(NCORES))
    ).results
    parts = [res[c]["out"].reshape(BL, 8, 8, D) for c in range(NCORES)]
    return np.concatenate(parts, axis=0)


def kernel_profiled(**inputs):
    """Like kernel() but requests an NTFF trace; returns (out, exec_time_ns, raw)."""
    nc = _get_nc(_with_bias(inputs))
    r = run_bass_kernel_spmd(
        nc, _make_in_maps(inputs), core_ids=list(range(NCORES)), trace=True
    )
    parts = [r.results[c]["out"].reshape(BL, 8, 8, D) for c in range(NCORES)]
    return np.concatenate(parts, axis=0), r.exec_time_ns, r


# revision 25
# speedup vs baseline: 1.0525x; 1.0525x over previous
"""Multi-head attention (B=512,S=64,D=1024,H=16) on 8 trn2 NeuronCores.

Strategy: pure data-parallel over the batch dim — each core gets 64 batches
(4096 tokens) and runs the full fused MHA layer locally; no collectives.

Per-core dataflow (token chunks of 512 = 8 batches):
  x [tok,1024] --PE transpose--> xT [1024,tok] (feature-major, bf16)
               --ACT scale-cast--> xT8 fp8 pair tiles [128,2,512] (x*16)
  qT/kT = fp8 DoubleRow GEMM (W*16 fp8 pair tiles), DVE evac scale 1/256
          -> bf16 feature-major m-tiles [128,512]
  v  = x @ Wv bf16 (token-major, interleaved with ones col in vaug)
  scoresT[k,q]: per (batch-pair u, t-pair) two psum banks, one per head
          parity (row-strip), each holding two heads' scores -> ONE exp
          ACT [128,128] per bank (exp/copy/identity share an ACT table;
          gelu does not, so gelus are clustered at chunk end)
  ctx[q,:]|sumexp[q] = expS.T @ [v|1] -> merged recip + one broadcast-mult TT
  ctxT via PE transpose; out = gelu(ctx @ Wo) accumulated token-major -> DRAM

fp8 notes: e4m3 needs pre-scaling (x16) to stay out of the subnormal range;
Q/K tolerate plain fp8 (softmax normalizes the small logit error), V/O do
not (W-quantization error hits the output directly), so V/O stay bf16.
DoubleRow sustains ~1.9x bf16 row throughput on HW. fp8 casts run on ACT
(Copy w/ scale); DVE/GPSIMD do fp8 stores 5-50x slower.

PSUM rules (hardware, probe-verified): concurrent matmuls may share a bank
only on the same row-strip or a strict diagonal; start=True zeroes the full
bank for the partitions it writes, so a second quadrant write on the same
partitions uses start=False, and sibling accumulation groups on the same
partitions must be strictly sequential (h-outer ordering in the DR GEMMs).
"""

import sys

sys.path.insert(0, "/opt/trn_rl_repo")

import numpy as np

import concourse.bass as bass
import concourse.tile as tile
from concourse import mybir
from concourse.bass_utils import run_bass_kernel_spmd
from concourse.masks import make_identity

F32 = mybir.dt.float32
BF = mybir.dt.bfloat16
F8 = mybir.dt.float8e4

B, S, D, H = 512, 64, 1024, 16
DH = D // H  # 64
NCORES = 8
BL = B // NCORES  # 64 batches per core
NTOK = BL * S  # 4096 tokens per core
CHUNK = 512  # tokens per pipeline chunk (8 batches)
NCH = NTOK // CHUNK  # 8
TT = CHUNK // 128  # 4 token-tiles per chunk
KT = D // 128  # 8 d-tiles
NP = KT // 2  # 4 fp8 k-pair tiles
SCALE = 1.0 / np.sqrt(np.float32(D))  # 1/32
XS = 16.0  # fp8 pre-scale for x
WS = 16.0  # fp8 pre-scale for wq/wk


def _split_multiwait(nc, limit=1):
    """walrus can emit at most one sync-wait per instruction; TileContext's
    tail drain carries one wait per touched processor. Hoist extras onto
    chained NOPs."""
    f = nc.m.functions[0]
    for blk in f.blocks:
        new_insts = []
        for inst in blk.instructions:
            si = inst.sync_info
            if si is not None and len(si.on_wait) > limit:
                extra = si.on_wait[:-limit]
                keep = si.on_wait[-limit:]
                for i, w in enumerate(extra):
                    nop = mybir.InstNoOp(
                        name=f"{inst.name}-waitsplit{i}",
                        sync_info=mybir.SyncInfo(on_wait=[w], on_update=[]),
                        bass_nofuse=True,
                        ins=[],
                        outs=[],
                    )
                    nop.engine = inst.engine
                    new_insts.append(nop)
                si.on_wait[:] = keep
            new_insts.append(inst)
        blk.instructions[:] = new_insts


def _interleave(a, b):
    """Merge two unit lists round-robin, proportionally to their lengths."""
    out = []
    ia = ib = 0
    la, lb = len(a), len(b)
    while ia < la or ib < lb:
        if ib >= lb or (ia < la and ia * lb <= ib * la):
            out.append(a[ia])
            ia += 1
        else:
            out.append(b[ib])
            ib += 1
    return out


def build(with_bias=False, split_waits=True):
    nc = bass.Bass("TRN2", debug=False, num_devices=NCORES)

    x_d = nc.declare_dram_parameter("x", [NTOK, D], F32, isOutput=False)
    w_d = {}
    b_d = {}
    for nm in ("wq", "wk", "wv", "wo"):
        w_d[nm] = nc.declare_dram_parameter(f"{nm}_w", [D, D], F32, isOutput=False)
        b_d[nm] = nc.declare_dram_parameter(f"{nm}_b", [D], F32, isOutput=False)
    out_d = nc.declare_dram_parameter("out", [NTOK, D], F32, isOutput=True)

    with tile.TileContext(nc) as tc:
        with (
            tc.tile_pool(name="weights", bufs=1) as wpool,
            tc.tile_pool(name="consts", bufs=1) as cpool,
            tc.tile_pool(name="wload", bufs=4) as ldpool,
            tc.tile_pool(name="xin", bufs=2) as xpool,
            tc.tile_pool(name="feat", bufs=2) as fpool,
            tc.tile_pool(name="attn", bufs=4) as apool,
            tc.tile_pool(name="outb", bufs=2) as opool,
            tc.tile_pool(name="psum", bufs=2, space="PSUM") as ppool,
        ):
            # fp8 pair tiles for Q/K: w8[which][p] = [128, 2, 8m, 128]
            w8 = {nm: [None] * NP for nm in ("wq", "wk")}
            # bf16 weights for V/O: wt[nm][k] = [128, D]
            wt = {nm: [None] * KT for nm in ("wv", "wo")}
            biases = {}
            consts = {}

            def pe_transpose(src, dst, evac=None):
                ps = ppool.tile([128, 128], BF, tag="tp", bufs=1, name="ps_tp")
                nc.tensor.transpose(ps, src, consts["identity"])
                (evac or nc.vector).tensor_copy(out=dst, in_=ps)

            def unit_load_w8(nm, k):
                """Load f32 ktile of wq/wk, ACT scale-cast into fp8 pair tiles."""
                def f():
                    wf = ldpool.tile([128, D], F32, tag="wload", name="wf")
                    nc.sync.dma_start(
                        out=wf[:], in_=w_d[nm][k * 128 : (k + 1) * 128, :]
                    )
                    if w8[nm][k // 2] is None:
                        w8[nm][k // 2] = wpool.tile(
                            [128, 2, KT, 128], F8, tag=f"w8_{nm}_{k // 2}",
                            name=f"w8{nm}{k // 2}",
                        )
                    nc.scalar.activation(
                        out=w8[nm][k // 2][:, k % 2, :, :],
                        in_=wf[:],
                        func=mybir.ActivationFunctionType.Copy,
                        scale=float(WS),
                    )

                return f

            def unit_load_w16(nm, k):
                def f():
                    wf = ldpool.tile([128, D], F32, tag="wload", name="wf")
                    nc.sync.dma_start(
                        out=wf[:], in_=w_d[nm][k * 128 : (k + 1) * 128, :]
                    )
                    wb = wpool.tile([128, D], BF, tag=f"w_{nm}_{k}", name=f"w{nm}{k}")
                    nc.vector.tensor_copy(out=wb[:], in_=wf[:])
                    wt[nm][k] = wb

                return f

            def unit_qk_biases():
                def f():
                    for nm in ("wq", "wk"):
                        bt = cpool.tile([128, KT], F32, tag=f"{nm}_pb", name=f"{nm}_pb")
                        nc.sync.dma_start(
                            out=bt[:], in_=b_d[nm][:].rearrange("(m p) -> p m", p=128)
                        )
                        biases[nm] = bt

                return f

            def unit_biases():
                """Only used in the with_bias build: v/o biases folded into a
                rank-1 bias_row = wv_b @ Wo + wo_b added inside the O GEMM."""
                def f():
                    vb = cpool.tile([128, KT], F32, tag="vb_pb", name="vb_pb")
                    nc.sync.dma_start(
                        out=vb[:], in_=b_d["wv"][:].rearrange("(m p) -> p m", p=128)
                    )
                    vbb = cpool.tile([128, KT], BF, tag="vb_pbb", name="vb_pbb")
                    nc.vector.tensor_copy(out=vbb[:], in_=vb[:])
                    ob = ldpool.tile([1, D], F32, tag="wload", name="ob")
                    nc.sync.dma_start(out=ob[:], in_=b_d["wo"][:].unsqueeze(0))
                    brow = cpool.tile([1, D], BF, tag="brow", name="brow")
                    for n in range(2):
                        psb = ppool.tile([1, 512], F32, tag="cx", bufs=2, name="psb")
                        for k in range(KT):
                            nc.tensor.matmul(
                                psb[:],
                                lhsT=vbb[:, k : k + 1],
                                rhs=wt["wo"][k][:, n * 512 : (n + 1) * 512],
                                start=(k == 0),
                                stop=(k == KT - 1),
                            )
                        nc.vector.tensor_tensor(
                            out=brow[:, n * 512 : (n + 1) * 512],
                            in0=psb[:],
                            in1=ob[:, n * 512 : (n + 1) * 512],
                            op=mybir.AluOpType.add,
                        )
                    biases["brow"] = brow
                    ones_col = cpool.tile([1, 128], BF, tag="ones_col", name="ones_col")
                    nc.gpsimd.memset(ones_col[:], 1.0)
                    biases["ones_col"] = ones_col

                return f

            live = {}  # per-chunk tiles handed from stage A to stage B

            def stage_a_units(ch):
                """X load + transpose + fp8 prep, then QKV projections."""
                tok0 = ch * CHUNK
                st = live.setdefault(ch, {})

                def u_x(t):
                    def f():
                        if "xT" not in st:
                            st["xT"] = fpool.tile(
                                [128, KT, CHUNK], BF, tag="xT", name="xT"
                            )
                        xf = xpool.tile([128, D], F32, tag="xf32", name="xf")
                        nc.sync.dma_start(
                            out=xf[:], in_=x_d[tok0 + t * 128 : tok0 + (t + 1) * 128, :]
                        )
                        xb = xpool.tile([128, D], BF, tag="xbf", name="xb")
                        nc.gpsimd.tensor_copy(out=xb[:], in_=xf[:])
                        for k in range(KT):
                            pe_transpose(
                                xb[:, k * 128 : (k + 1) * 128],
                                st["xT"][:, k, t * 128 : (t + 1) * 128],
                            )

                    return f

                def u_x8(p):
                    def f():
                        if "xT8" not in st:
                            st["xT8"] = [
                                fpool.tile(
                                    [128, 2, CHUNK], F8, tag=f"xT8{i}", name=f"xT8{i}"
                                )
                                for i in range(NP)
                            ]
                        for i in range(2):
                            nc.scalar.activation(
                                out=st["xT8"][p][:, i, :],
                                in_=st["xT"][:, 2 * p + i, :],
                                func=mybir.ActivationFunctionType.Copy,
                                scale=float(XS),
                            )

                    return f

                def u_qk(which, m):
                    def f():
                        key = "qT" if which == "wq" else "kT"
                        if key not in st:
                            st[key] = [
                                fpool.tile([128, CHUNK], BF, tag=f"{key}{i}",
                                           name=f"{key}{i}")
                                for i in range(KT)
                            ]
                        ps = ppool.tile([128, CHUNK], F32, tag="proj", bufs=3, name="ps_qk")
                        for h in range(2):
                            for p in range(NP):
                                nc.tensor.matmul(
                                    ps[:, h * 256 : (h + 1) * 256],
                                    lhsT=w8[which][p][:, :, m, :],
                                    rhs=st["xT8"][p][:, :, h * 256 : (h + 1) * 256],
                                    start=(p == 0),
                                    stop=(p == NP - 1),
                                    perf_mode=mybir.MatmulPerfMode.DoubleRow,
                                )
                        if with_bias:
                            nc.scalar.activation(
                                out=st[key][m][:],
                                in_=ps[:],
                                func=mybir.ActivationFunctionType.Identity,
                                scale=float(1.0 / (XS * WS)),
                                bias=biases[which][:, m : m + 1],
                            )
                        else:
                            nc.scalar.activation(
                                out=st[key][m][:],
                                in_=ps[:],
                                func=mybir.ActivationFunctionType.Copy,
                                scale=float(1.0 / (XS * WS)),
                            )

                    return f

                def u_v(t, n):
                    def f():
                        if "vaug" not in st:
                            st["vaug"] = [
                                apool.tile(
                                    [128, H * (DH + 1)], BF,
                                    tag=f"vaug{i}", name=f"vaug{i}", bufs=2,
                                )
                                for i in range(TT)
                            ]
                            for i in range(TT):
                                nc.gpsimd.memset(
                                    st["vaug"][i][:]
                                    .rearrange("p (h c) -> p h c", c=DH + 1)[:, :, DH : DH + 1],
                                    1.0,
                                )
                        ps = ppool.tile([128, CHUNK], F32, tag="proj", bufs=3, name="ps_v")
                        for k in range(KT):
                            nc.tensor.matmul(
                                ps[:],
                                lhsT=st["xT"][:, k, t * 128 : (t + 1) * 128],
                                rhs=wt["wv"][k][:, n * 512 : (n + 1) * 512],
                                start=(k == 0),
                                stop=(k == KT - 1),
                            )
                        nc.vector.tensor_copy(
                            out=st["vaug"][t][:]
                            .rearrange("p (h c) -> p h c", c=DH + 1)[:, n * 8 : (n + 1) * 8, 0:DH],
                            in_=ps[:].rearrange("p (j c) -> p j c", c=DH),
                        )

                    return f

                proj = []
                for m in range(KT):
                    proj.append(u_qk("wq", m))
                    proj.append(u_qk("wk", m))
                for t in range(TT):
                    for n in range(2):
                        proj.append(u_v(t, n))
                return {
                    "x": [u_x(t) for t in range(TT)],
                    "x8": [u_x8(p) for p in range(NP)],
                    "q": [u_qk("wq", m) for m in range(KT)],
                    "k": [u_qk("wk", m) for m in range(KT)],
                    "v": [u_v(t, n) for t in range(TT) for n in range(2)],
                    "proj": proj,
                }

            def attn_units(ch):
                """Attention for chunk ch. Scores for a (u, t-pair) fill two
                psum banks (one per head parity = row strip), two heads per
                bank, one exp each. Gelu/out units are clustered at the end
                of the chunk to avoid ACT table thrash."""
                st = live[ch]
                es_tiles = {}

                def u_scores(u, tq):
                    def f():
                        qT, kT = st["qT"], st["kT"]
                        sc = {}
                        for hh in (0, 1):
                            sc[hh] = ppool.tile(
                                [128, 256], F32, tag=f"sc{hh}",
                                bufs=1, name=f"ps_sc{hh}",
                            )
                        for hh in (0, 1):
                            hsl = slice(hh * 64, hh * 64 + 64)
                            for dt in range(4):
                                t = 4 * tq + dt
                                for bpar in (0, 1):
                                    toksl = slice(
                                        u * 128 + bpar * 64, u * 128 + bpar * 64 + 64
                                    )
                                    nc.tensor.matmul(
                                        sc[hh][
                                            bpar * 64 : bpar * 64 + 64,
                                            dt * 64 : dt * 64 + 64,
                                        ],
                                        lhsT=kT[t][hsl, toksl],
                                        rhs=qT[t][hsl, toksl],
                                        start=(dt == 0),
                                        stop=(dt == 3),
                                        skip_group_check=True,
                                    )
                        for hh in (0, 1):
                            es = apool.tile([128, 256], BF, tag="expS", name="es", bufs=8)
                            es_tiles[(u, tq, hh)] = es
                            nc.scalar.activation(
                                out=es[:],
                                in_=sc[hh][:],
                                func=mybir.ActivationFunctionType.Exp,
                                scale=float(SCALE),
                            )

                    return f

                def u_ctx(u, t):
                    def f():
                        if "ctx" not in st:
                            st["ctx"] = [
                                apool.tile([128, D], BF, tag=f"ctx{i}", name=f"ctx{i}", bufs=2)
                                for i in range(TT)
                            ]
                        vaug, ctx = st["vaug"], st["ctx"]
                        tq, dt = t // 4, t % 4
                        ps_c = ppool.tile([128, 130], F32, tag="cx", bufs=2, name="ps_c")
                        for bpar in (0, 1):
                            bsl = slice(bpar * 64, bpar * 64 + 64)
                            for hh in (0, 1):
                                h = 2 * t + hh
                                es = es_tiles[(u, tq, hh)]
                                nc.tensor.matmul(
                                    ps_c[bsl, hh * 65 : hh * 65 + 65],
                                    lhsT=es[bsl, dt * 64 : dt * 64 + 64],
                                    rhs=vaug[u][bsl, h * 65 : (h + 1) * 65],
                                    start=True,
                                    stop=True,
                                )
                        if dt == 3:
                            for hh in (0, 1):
                                es_tiles.pop((u, tq, hh))
                        rc2 = apool.tile([128, 2], F32, tag="recip", name="rc2")
                        psv = ps_c[:].rearrange("p (h c) -> p h c", c=65)
                        nc.vector.reciprocal(rc2[:], psv[:, :, DH])
                        nc.vector.tensor_tensor(
                            out=ctx[u][:, 2 * t * DH : (2 * t + 2) * DH].rearrange(
                                "p (h c) -> p h c", c=DH
                            ),
                            in0=psv[:, :, 0:DH],
                            in1=rc2[:, :, None].to_broadcast([128, 2, DH]),
                            op=mybir.AluOpType.mult,
                        )

                    return f

                def u_ctxT(u):
                    def f():
                        if "cT" not in st:
                            st["cT"] = fpool.tile(
                                [128, KT, CHUNK], BF, tag="cT", name="cT"
                            )
                        for k in range(KT):
                            pe_transpose(
                                st["ctx"][u][:, k * 128 : (k + 1) * 128],
                                st["cT"][:, k, u * 128 : (u + 1) * 128],
                            )

                    return f

                def u_out(t, n):
                    tok0 = ch * CHUNK

                    def f():
                        ps = ppool.tile([128, CHUNK], F32, tag="proj", bufs=3, name="ps_o")
                        for k in range(KT):
                            nc.tensor.matmul(
                                ps[:],
                                lhsT=st["cT"][:, k, t * 128 : (t + 1) * 128],
                                rhs=wt["wo"][k][:, n * 512 : (n + 1) * 512],
                                start=(k == 0),
                                stop=(k == KT - 1) and not with_bias,
                            )
                        if with_bias:
                            nc.tensor.matmul(
                                ps[:],
                                lhsT=biases["ones_col"][:],
                                rhs=biases["brow"][:, n * 512 : (n + 1) * 512],
                                start=False,
                                stop=True,
                            )
                        og = opool.tile([128, 512], F32, tag="ogelu", name="og", bufs=4)
                        nc.scalar.activation(
                            out=og[:], in_=ps[:], func=mybir.ActivationFunctionType.Gelu
                        )
                        nc.sync.dma_start(
                            out=out_d[
                                tok0 + t * 128 : tok0 + (t + 1) * 128,
                                n * 512 : (n + 1) * 512,
                            ],
                            in_=og[:],
                        )

                    return f

                units = []
                for u in range(TT):
                    units.append(u_scores(u, 0))
                    if u > 0:
                        units.append(u_ctxT(u - 1))
                    units.append(u_ctx(u, 0))
                    units.append(u_ctx(u, 1))
                    units.append(u_scores(u, 1))
                    for t in range(2, KT):
                        units.append(u_ctx(u, t))
                units.append(u_ctxT(TT - 1))
                # gelu cluster: all out-projections of the chunk back to back
                for u in range(TT):
                    units.append(u_out(u, 0))
                    units.append(u_out(u, 1))
                return units

            # ---- emission ----
            identity = cpool.tile([128, 128], BF, tag="ident", name="identity")
            make_identity(nc, identity[:])
            consts["identity"] = identity
            stages = [stage_a_units(ch) for ch in range(NCH)]
            # prologue: x(0), wq8 loads, x8(0), then chunk-0 Q interleaved with
            # wk8 loads and x(1); K with wv loads; V with wo loads.
            for t in range(TT):
                stages[0]["x"][t]()
            for p in range(NP):
                stages[0]["x8"][p]()
            if with_bias:
                unit_qk_biases()()
            for k in range(KT):
                unit_load_w8("wq", k)()
            for k in range(KT):
                unit_load_w8("wk", k)()
            for u in _interleave(stages[0]["q"], stages[1]["x"][:2]):
                u()
            for u in _interleave(
                _interleave(stages[0]["k"], stages[1]["x"][2:]),
                [unit_load_w16("wv", k) for k in range(KT)],
            ):
                u()
            for u in _interleave(
                stages[0]["v"],
                [unit_load_w16("wo", k) for k in range(KT)],
            ):
                u()
            if with_bias:
                unit_biases()()
            # steady state: block ch emits x8(ch) + proj(ch) + x(ch+1) + attn(ch-1)
            for ch in range(1, NCH):
                dense = stages[ch]["x8"] + stages[ch]["proj"]
                if ch + 1 < NCH:
                    dense = _interleave(dense, stages[ch + 1]["x"])
                for u in _interleave(dense, attn_units(ch - 1)):
                    u()
                live.pop(ch - 1)
            for u in attn_units(NCH - 1):
                u()
            live.pop(NCH - 1)

    if split_waits:
        _split_multiwait(nc)
    return nc


_NC = {}


def _get_nc(with_bias):
    if with_bias not in _NC:
        _NC[with_bias] = build(with_bias=with_bias)
    return _NC[with_bias]


def _make_in_maps(inputs):
    x = np.ascontiguousarray(np.asarray(inputs["x"], dtype=np.float32))
    full = {
        nm: np.ascontiguousarray(np.asarray(inputs[nm], dtype=np.float32))
        for nm in ("wq_w", "wq_b", "wk_w", "wk_b", "wv_w", "wv_b", "wo_w", "wo_b")
    }
    in_maps = []
    for c in range(NCORES):
        m = {"x": np.ascontiguousarray(x[c * BL : (c + 1) * BL].reshape(NTOK, D))}
        m.update(full)
        in_maps.append(m)
    return in_maps


def _with_bias(inputs):
    return any(
        np.abs(np.asarray(inputs[nm])).max() > 0
        for nm in ("wq_b", "wk_b", "wv_b", "wo_b")
    )


def kernel(**inputs):
    nc = _get_nc(_with_bias(inputs))
    res = run_bass_kernel_spmd(
        nc, _make_in_maps(inputs), core_ids=list(range(NCORES))
    ).results
    parts = [res[c]["out"].reshape(BL, 8, 8, D) for c in range(NCORES)]
    return np.concatenate(parts, axis=0)


def kernel_profiled(**inputs):
    """Like kernel() but requests an NTFF trace; returns (out, exec_time_ns, raw)."""
    nc = _get_nc(_with_bias(inputs))
    r = run_bass_kernel_spmd(
        nc, _make_in_maps(inputs), core_ids=list(range(NCORES)), trace=True
    )
    parts = [r.results[c]["out"].reshape(BL, 8, 8, D) for c in range(NCORES)]
    return np.concatenate(parts, axis=0), r.exec_time_ns, r
